# revision 1
# baseline (speedup 1.0000x reference)
"""GQA multi-head attention (B=4, S=2048, D=576, 9 Q heads / 3 KV heads,
causal) for 8 Trainium2 NeuronCores.

Sharding: 2 cores per batch item, split over the query dimension in
causally-balanced quarter pairs:
  type A core: q rows [0:512) + [1536:2048)   (kb counts 2,4,14,16 per 256-sb)
  type B core: q rows [512:1536)              (kb counts 6,8,10,12)
Each core redundantly computes K/V projections for the keys it needs.
Two compiled programs (A and B), 4 cores each.

Layout strategy (everything "transposed", d_model on partitions):
  XT_ext [577, Skv]  (row 576 = ones, folds biases into projections)
  QT     [576, 1024] (head h at chunk h//2, partition (h%2)*64)
  KT     [192, Skv] + duplicated halves for row-packed score matmuls
  V      [Skv, 3, 65] natural (65th col = ones -> softmax denominator)
  scores^T [k, q] tiles -> exp (ACT) -> causal mask (gpsimd affine_select)
  attn@V: O^T[65, 256q] psum accumulation over k blocks
  normalize: reciprocal of row 64, gpsimd partition_broadcast, DVE mul
  out-proj: lhsT = attnT chunks (ones row folds bo), out rows = q
All matmuls fp32r (full PE rate at free dim >= 256); V projection bf16.
"""

import numpy as np
import ml_dtypes

import concourse.bass as bass
import concourse.bacc as bacc
import concourse.tile as tile
from concourse import mybir
from concourse.bass_utils import run_bass_kernel_spmd

F32 = mybir.dt.float32
F32R = mybir.dt.float32r
BF16 = mybir.dt.bfloat16

B, S, DM = 4, 2048, 576
DME = DM + 1          # + ones row for bias folding
HD = 64               # head dim
NH = 9                # query heads
NKV = 3               # kv heads
SB = 256              # q superblock (free dim of score matmuls)
NSB = 4               # q superblocks per core (1024 q rows)
SQ = SB * NSB
G = 4                 # kb-blocks per exp batch ([128, 1024] psum)

# contraction chunks over DME=577: 4x128 + 65
CHUNKS = [(0, 128), (128, 128), (256, 128), (384, 128), (512, 65)]
# M chunks over 576 outputs: 4x128 + 64
MCHUNKS = [(0, 128), (128, 128), (256, 128), (384, 128), (512, 64)]

TYPE_A = dict(qs=[0, 256, 1536, 1792], skv=2048)
TYPE_B = dict(qs=[512, 768, 1024, 1280], skv=1536)

# head pairs for row-packed score matmuls: (head at array rows 0:64,
# head at rows 64:128); head 8 runs solo.
PAIRS = [(0, 1), (2, 3), (4, 5), (6, 7), (8,)]


def kv_of(h):
    return h // NKV


def _r(ap):
    return ap.bitcast(F32R)


def build_program(qs_list, skv, reps=1, no_afsel=False, no_bcast=False, loop_reps=0):
    nc = bacc.Bacc("TRN2", target_bir_lowering=False, debug=False, num_devices=4)
    nkb = skv // 128

    xt_d = nc.dram_tensor("xt", [DME, skv], F32R, kind="ExternalInput")
    wq_d = nc.dram_tensor("wq", [DME, DM], F32R, kind="ExternalInput")
    wk_d = nc.dram_tensor("wk", [DME, NKV * HD], F32R, kind="ExternalInput")
    wv_d = nc.dram_tensor("wv", [DME, NKV * HD], BF16, kind="ExternalInput")
    wo_d = nc.dram_tensor("wo", [DME, DM], F32R, kind="ExternalInput")
    ones_d = nc.dram_tensor("ones", [128, 9], F32R, kind="ExternalInput")
    out_d = nc.dram_tensor("out", [SQ, DM], F32, kind="ExternalOutput")

    import contextlib
    with tile.TileContext(nc) as tc:
      for _rep in range(reps):
       with (tc.For_i(0, loop_reps, 1) if loop_reps else contextlib.nullcontext()):
        with (
            tc.tile_pool(name="const", bufs=1) as constp,
            tc.tile_pool(name="proj", bufs=1) as projp,
            tc.tile_pool(name="attn", bufs=1) as attnp,
        ):
            # ---- weight loads ----
            wq_sb, wk_sb, wv_sb, wo_sb = [], [], [], []
            for c, (r0, pc) in enumerate(CHUNKS):
                t = constp.tile([pc, DM], F32R, name=f"wq{c}", tag=f"wq{c}")
                nc.sync.dma_start(t, wq_d[r0:r0 + pc, :])
                wq_sb.append(t)
                t = constp.tile([pc, NKV * HD], F32R, name=f"wk{c}", tag=f"wk{c}")
                nc.sync.dma_start(t, wk_d[r0:r0 + pc, :])
                wk_sb.append(t)
                t = constp.tile([pc, NKV * HD], BF16, name=f"wv{c}", tag=f"wv{c}")
                nc.sync.dma_start(t, wv_d[r0:r0 + pc, :])
                wv_sb.append(t)
                t = constp.tile([pc, DM], F32R, name=f"wo{c}", tag=f"wo{c}")
                nc.sync.dma_start(t, wo_d[r0:r0 + pc, :])
                wo_sb.append(t)

            with (
                tc.tile_pool(name="xpool", bufs=1) as xp,
                tc.tile_pool(name="xbfpool", bufs=1) as xbfp,
                tc.tile_pool(name="psproj", bufs=4, space="PSUM") as psproj,
            ):
                # ---- XT load + bf16 copy ----
                xt_sb, xtbf_sb = [], []
                for c, (r0, pc) in enumerate(CHUNKS):
                    t = xp.tile([pc, skv], F32R, name=f"xt{c}", tag=f"xt{c}")
                    nc.sync.dma_start(t, xt_d[r0:r0 + pc, :])
                    xt_sb.append(t)
                    tb = xbfp.tile([pc, skv], BF16, name=f"xtbf{c}", tag=f"xtbf{c}")
                    nc.vector.tensor_copy(tb, t)
                    xtbf_sb.append(tb)

                # ---- V projection (bf16) -> V [128, nkb, 3, 65], col 64 = 1.0
                vall = projp.tile([128, nkb, NKV, HD + 1], F32R, name="vall")
                nc.sync.dma_start(
                    vall[:, :, :, HD:HD + 1].rearrange("p a b c -> p (a b c)"),
                    bass.AP(ones_d, 0, [[9, 128], [0, nkb * NKV]]),
                )
                for sblk in range(nkb):
                    psf = psproj.tile([128, 512], F32, name="psv", tag="psp")
                    ps = psf[:, 0:NKV * HD]
                    for c, (r0, pc) in enumerate(CHUNKS):
                        nc.tensor.matmul(
                            ps,
                            xtbf_sb[c][:, sblk * 128:(sblk + 1) * 128],
                            wv_sb[c],
                            start=(c == 0), stop=(c == len(CHUNKS) - 1),
                        )
                    nc.scalar.activation(
                        vall[:, sblk, :, 0:HD],
                        ps.rearrange("p (h d) -> p h d", h=NKV),
                        func=mybir.ActivationFunctionType.Copy,
                    )

                # ---- QT projection -> qt[c] [128|64, 1024] ----
                qt_sb = []
                for c, (m0, mp) in enumerate(MCHUNKS):
                    t = projp.tile([mp, SQ], F32R, name=f"qt{c}", tag=f"qt{c}")
                    qt_sb.append(t)
                    for nb in range(2):  # two 512-col q blocks, contiguous globally
                        qg = qs_list[2 * nb]
                        ps = psproj.tile([128, 512], F32, name="psq", tag="psp")
                        for k, (r0, pc) in enumerate(CHUNKS):
                            nc.tensor.matmul(
                                ps[:mp, :],
                                (wq_sb[k][:, m0:m0 + mp]),
                                (xt_sb[k][:, qg:qg + 512]),
                                start=(k == 0), stop=(k == len(CHUNKS) - 1),
                            )
                        nc.scalar.activation(t[:, nb * 512:(nb + 1) * 512], ps[:mp, :],
                                             func=mybir.ActivationFunctionType.Copy)

                # ---- KT projection -> kt0 [128, skv] (kv0;kv1), kt1 [64, skv]
                kt_sb = []
                for ci, (m0, mp) in enumerate([(0, 128), (128, 64)]):
                    t = projp.tile([mp, skv], F32R, name=f"kt{ci}", tag=f"kt{ci}")
                    kt_sb.append(t)
                    for nb in range(skv // 512):
                        ps = psproj.tile([128, 512], F32, name="psk", tag="psp")
                        for k, (r0, pc) in enumerate(CHUNKS):
                            nc.tensor.matmul(
                                ps[:mp, :],
                                (wk_sb[k][:, m0:m0 + mp]),
                                (xt_sb[k][:, nb * 512:(nb + 1) * 512]),
                                start=(k == 0), stop=(k == len(CHUNKS) - 1),
                            )
                        nc.scalar.activation(t[:, nb * 512:(nb + 1) * 512], ps[:mp, :],
                                             func=mybir.ActivationFunctionType.Copy)

                # duplicated KT halves so every score pair can row-pack:
                # dup0 = [kv1 ; kv0], dup1[64:128] = kv2
                dup0 = projp.tile([128, skv], F32R, name="dup0")
                nc.vector.tensor_copy(dup0[0:64, :], kt_sb[0][64:128, :])
                nc.vector.tensor_copy(dup0[64:128, :], kt_sb[0][0:64, :])
                dup1 = projp.tile([128, skv], F32R, name="dup1")
                nc.vector.tensor_copy(dup1[64:128, :], kt_sb[1][0:64, :])

            # KT slice for kv head at a given array-half (base 0 or 64)
            def kt_src(kv, base):
                if base == 0:
                    return [kt_sb[0][0:64, :], dup0[0:64, :], kt_sb[1][0:64, :]][kv]
                return [dup0[64:128, :], kt_sb[0][64:128, :], dup1[64:128, :]][kv]

            # attnT chunks [128, 1024] (+ ones row on chunk 4 for bo folding)
            at_sb = []
            for c in range(4):
                t = attnp.tile([128, SQ], F32R, name=f"at{c}", tag=f"at{c}")
                at_sb.append(t)
            t = attnp.tile([65, SQ], F32R, name="at4", tag="at4")
            nc.sync.dma_start(
                t[64:65, :],
                bass.AP(ones_d, 0, [[0, 1], [1, SQ]]),
            )
            at_sb.append(t)

            # ---- attention ----
            with (
                tc.tile_pool(name="pssc", bufs=3, space="PSUM") as pssc,
                tc.tile_pool(name="psot", bufs=2, space="PSUM") as psot,
                tc.tile_pool(name="wtpool", bufs=5) as wtp,
                tc.tile_pool(name="small", bufs=8) as smallp,
            ):
                for sb in range(NSB):
                    qs = qs_list[sb]
                    nkbs = qs // 128 + 2
                    for pair in PAIRS:
                        ots = []
                        for h in pair:
                            ot = psot.tile([65, SB], F32, name="ot", tag="ot")
                            ots.append(ot)
                        qt_c = qt_sb[pair[0] // 2]
                        for g0 in range(0, nkbs, G):
                            gn = min(G, nkbs - g0)
                            pss, wts = [], []
                            for i, h in enumerate(pair):
                                base = (h % 2) * 64
                                ps = pssc.tile([128, G * SB], F32, name="pssc", tag="pssc")
                                wt = wtp.tile([128, G * SB], F32R, name="wt", tag="wt")
                                pss.append(ps)
                                wts.append(wt)
                                lhs = kt_src(kv_of(h), base)
                                rhs = qt_c[base:base + 64, sb * SB:(sb + 1) * SB]
                                for j in range(gn):
                                    kb = g0 + j
                                    nc.tensor.matmul(
                                        ps[:, j * SB:(j + 1) * SB],
                                        (lhs[:, kb * 128:(kb + 1) * 128]),
                                        (rhs),
                                        start=True, stop=True,
                                        tile_position=(base, 0),
                                    )
                                # exp(scores/8), psum -> sbuf
                                nc.scalar.activation(
                                    wt[:, 0:gn * SB], ps[:, 0:gn * SB],
                                    func=mybir.ActivationFunctionType.Exp,
                                    scale=0.125,
                                )
                                # causal mask on the (up to 2) diagonal blocks
                                for j in range(gn):
                                    kb = g0 + j
                                    if kb >= nkbs - 2 and not no_afsel:
                                        nc.gpsimd.affine_select(
                                            out=wt[:, j * SB:(j + 1) * SB],
                                            in_=wt[:, j * SB:(j + 1) * SB],
                                            pattern=[[1, SB]],
                                            compare_op=mybir.AluOpType.is_ge,
                                            fill=0.0,
                                            base=qs - kb * 128,
                                            channel_multiplier=-1,
                                        )
                            # attn @ V accumulation
                            for i, h in enumerate(pair):
                                for j in range(gn):
                                    kb = g0 + j
                                    nc.tensor.matmul(
                                        ots[i],
                                        (vall[:, kb, kv_of(h), :]),
                                        (wts[i][:, j * SB:(j + 1) * SB]),
                                        start=(kb == 0), stop=(kb == nkbs - 1),
                                    )
                        # normalize + evacuate to attnT
                        for i, h in enumerate(pair):
                            rec = smallp.tile([1, SB], F32, name="rec", tag="rec")
                            nc.vector.reciprocal(rec, ots[i][64:65, :])
                            bc = smallp.tile([64, SB], F32, name="bc", tag="bc")
                            if no_bcast:
                                nc.vector.memset(bc, 1.0)
                            else:
                                nc.gpsimd.partition_broadcast(bc, rec)
                            r0 = (h % 2) * 64
                            nc.vector.tensor_mul(
                                at_sb[h // 2][r0:r0 + 64, sb * SB:(sb + 1) * SB],
                                ots[i][0:64, :],
                                bc,
                            )

            # ---- output projection ----
            with (
                tc.tile_pool(name="psout", bufs=4, space="PSUM") as psout,
                tc.tile_pool(name="outpool", bufs=4) as outp,
            ):
                for qb in range(SQ // 128):
                    ot = outp.tile([128, DM], F32, name="outt", tag="outt")
                    for half in range(2):
                        ps = psout.tile([128, 288], F32, name="pso", tag="pso")
                        for c in range(5):
                            lhsT = at_sb[c][:, qb * 128:(qb + 1) * 128]
                            nc.tensor.matmul(
                                ps,
                                (lhsT),
                                (wo_sb[c][:, half * 288:(half + 1) * 288]),
                                start=(c == 0), stop=(c == 4),
                            )
                        nc.vector.tensor_copy(ot[:, half * 288:(half + 1) * 288], ps)
                    nc.sync.dma_start(out_d[qb * 128:(qb + 1) * 128, :], ot)

    nc.compile()
    return nc


_cache = {}


def _programs():
    if "A" not in _cache:
        _cache["A"] = build_program(TYPE_A["qs"], TYPE_A["skv"])
        _cache["B"] = build_program(TYPE_B["qs"], TYPE_B["skv"])
    return _cache["A"], _cache["B"]


def _make_runner(nc, devices):
    """Cached shard_map runner for `nc` pinned to an explicit device subset.

    Mirrors concourse.bass2jax.run_bass_via_pjrt's multi-core branch, but
    with a persistent jit and a caller-chosen device list so two programs
    can run concurrently on disjoint NeuronCore subsets.
    """
    import jax
    from jax.experimental.shard_map import shard_map
    from jax.sharding import Mesh, PartitionSpec
    from concourse import bass2jax, mybir as _mb

    bass2jax.install_neuronx_cc_hook()
    n_cores = len(devices)

    part_name = nc.partition_id_tensor.name if nc.partition_id_tensor else None
    in_names, out_names, out_avals = [], [], []
    for alloc in nc.m.functions[0].allocations:
        if not isinstance(alloc, mybir.MemoryLocationSet):
            continue
        name = alloc.memorylocations[0].name
        if alloc.kind == "ExternalInput":
            if name != part_name:
                in_names.append(name)
        elif alloc.kind == "ExternalOutput":
            out_names.append(name)
            out_avals.append(
                jax.core.ShapedArray(tuple(alloc.tensor_shape), _mb.dt.np(alloc.dtype))
            )
    n_params = len(in_names)
    n_outs = len(out_avals)
    all_names = in_names + out_names + ([part_name] if part_name else [])
    donate = tuple(range(n_params, n_params + n_outs))

    def _body(*args):
        args = list(args)
        if part_name:
            args.append(bass2jax.partition_id_tensor())
        outs = bass2jax._bass_exec_p.bind(
            *args,
            out_avals=tuple(out_avals),
            in_names=tuple(all_names),
            out_names=tuple(out_names),
            lowering_input_output_aliases=(),
            sim_require_finite=True,
            sim_require_nnan=True,
            nc=nc,
        )
        return tuple(outs)

    mesh = Mesh(np.asarray(devices), ("core",))
    in_specs = (PartitionSpec("core"),) * (n_params + n_outs)
    out_specs = (PartitionSpec("core"),) * n_outs
    sharded = jax.jit(
        shard_map(_body, mesh=mesh, in_specs=in_specs, out_specs=out_specs,
                  check_rep=False),
        donate_argnums=donate, keep_unused=True,
    )

    def run(in_maps, block=True):
        concat_in = [
            np.concatenate([np.asarray(m[name]) for m in in_maps], axis=0)
            for name in in_names
        ]
        zeros = [
            np.zeros((n_cores * a.shape[0], *a.shape[1:]), a.dtype) for a in out_avals
        ]
        out_arrs = sharded(*concat_in, *zeros)
        def collect():
            return [
                {name: np.asarray(out_arrs[i]).reshape(n_cores, *out_avals[i].shape)[c]
                 for i, name in enumerate(out_names)}
                for c in range(n_cores)
            ]
        return collect() if block else collect

    return run


def _runners():
    if "runA" not in _cache:
        import jax
        devs = jax.devices()
        ncA, ncB = _programs()
        _cache["runA"] = _make_runner(ncA, devs[0:4])
        _cache["runB"] = _make_runner(ncB, devs[4:8])
    return _cache["runA"], _cache["runB"]


def _host_inputs(inputs, Wq, bq, Wk, bk, Wv, bv, Wo, bo):
    x = np.asarray(inputs, dtype=np.float32)
    wq = np.vstack([np.asarray(Wq, np.float32), np.asarray(bq, np.float32)[None]])
    wk = np.vstack([np.asarray(Wk, np.float32), np.asarray(bk, np.float32)[None]])
    wv = np.vstack([np.asarray(Wv, np.float32), np.asarray(bv, np.float32)[None]]).astype(ml_dtypes.bfloat16)
    wo = np.vstack([np.asarray(Wo, np.float32), np.asarray(bo, np.float32)[None]])
    xts = []
    for b in range(B):
        xt = np.empty((DME, S), np.float32)
        xt[:DM] = x[b].T
        xt[DM] = 1.0
        xts.append(np.ascontiguousarray(xt))
    return xts, wq, wk, wv, wo


def kernel(inputs, Wq, bq, Wk, bk, Wv, bv, Wo, bo):
    xts, wq, wk, wv, wo = _host_inputs(inputs, Wq, bq, Wk, bk, Wv, bv, Wo, bo)
    ones = np.ones((128, 9), np.float32)
    maps_a = [dict(xt=xts[b], wq=wq, wk=wk, wv=wv, wo=wo, ones=ones)
              for b in range(B)]
    maps_b = [dict(xt=np.ascontiguousarray(xts[b][:, :TYPE_B["skv"]]), wq=wq,
                   wk=wk, wv=wv, wo=wo, ones=ones) for b in range(B)]
    try:
        run_a, run_b = _runners()
        col_a = run_a(maps_a, block=False)
        col_b = run_b(maps_b, block=False)
        res_a, res_b = col_a(), col_b()
    except Exception:
        res_a, res_b = _kernel_fallback(maps_a, maps_b)

    out = np.empty((B, S, DM), np.float32)
    for b in range(B):
        oa = res_a[b]["out"]
        ob = res_b[b]["out"]
        out[b, 0:512] = oa[0:512]
        out[b, 1536:2048] = oa[512:1024]
        out[b, 512:1536] = ob
    return out


def _kernel_fallback(maps_a, maps_b):
    ncA, ncB = _programs()
    res_a = run_bass_kernel_spmd(ncA, maps_a, core_ids=[0, 1, 2, 3]).results
    res_b = run_bass_kernel_spmd(ncB, maps_b, core_ids=[0, 1, 2, 3]).results
    return res_a, res_b


def _kernel_old(inputs, Wq, bq, Wk, bk, Wv, bv, Wo, bo):
    ncA, ncB = _programs()
    xts, wq, wk, wv, wo = _host_inputs(inputs, Wq, bq, Wk, bk, Wv, bv, Wo, bo)

    ones = np.ones((128, 9), np.float32)
    maps_a = [
        dict(xt=xts[b], wq=wq, wk=wk, wv=wv, wo=wo, ones=ones) for b in range(B)
    ]
    maps_b = [
        dict(xt=np.ascontiguousarray(xts[b][:, :TYPE_B["skv"]]), wq=wq, wk=wk, wv=wv, wo=wo, ones=ones)
        for b in range(B)
    ]

    res_a = run_bass_kernel_spmd(ncA, maps_a, core_ids=[0, 1, 2, 3]).results
    res_b = run_bass_kernel_spmd(ncB, maps_b, core_ids=[0, 1, 2, 3]).results

    out = np.empty((B, S, DM), np.float32)
    for b in range(B):
        oa = res_a[b]["out"]
        ob = res_b[b]["out"]
        out[b, 0:512] = oa[0:512]
        out[b, 1536:2048] = oa[512:1024]
        out[b, 512:1536] = ob
    return out



# revision 11
# speedup vs baseline: 3.7197x; 3.7197x over previous
"""GQA multi-head attention (B=4, S=2048, D=576, 9 Q heads / 3 KV heads,
causal) for 8 Trainium2 NeuronCores.

Sharding: 2 cores per batch item, split over the query dimension in
causally-balanced quarter pairs:
  type A core: q rows [0:512) + [1536:2048)   (kb counts 2,4,14,16 per 256-sb)
  type B core: q rows [512:1536)              (kb counts 6,8,10,12)
Each core redundantly computes K/V projections for the keys it needs.
Two compiled programs (A and B), 4 cores each.

Layout strategy (everything "transposed", d_model on partitions):
  XT_ext [577, Skv]  (row 576 = ones, folds biases into projections)
  QT     [576, 1024] (head h at chunk h//2, partition (h%2)*64)
  KT     [192, Skv] + duplicated halves for row-packed score matmuls
  V      [Skv, 3, 65] bf16 (65th col = ones -> softmax denominator)
  scores^T [k, q] psum shared per head pair -> one exp (ACT, bf16 out)
  -> causal mask via one DVE multiply with a constant 0/1 tile (diagonal
  blocks only) -> attn@V psum [65, 512] per pair accumulated over k
  normalize: DVE reciprocal of row 64, gpsimd partition_broadcast, DVE mul
  out-proj: per superblock; lhsT = attnT chunks (ones row folds bo)
K/V projection passes not needed by the first two superblocks are
interleaved into the attention loop to fill tensor-engine idle time.
Score/Q/K/out matmuls fp32r (full rate at free dim >= 256); V projection
and softmax weights bf16 (fp32r is quarter-rate below free dim 256).
"""

import numpy as np
import ml_dtypes

import concourse.bass as bass
import concourse.bacc as bacc
import concourse.tile as tile
from concourse import mybir
from concourse.bass_utils import run_bass_kernel_spmd

F32 = mybir.dt.float32
F32R = mybir.dt.float32r
BF16 = mybir.dt.bfloat16

B, S, DM = 4, 2048, 576
DME = DM + 1          # + ones row for bias folding
HD = 64               # head dim
NH = 9                # query heads
NKV = 3               # kv heads
SB = 256              # q superblock (free dim of score matmuls)
NSB = 4               # q superblocks per core (1024 q rows)
SQ = SB * NSB
G = 2                 # kb-blocks per head per exp batch ([128, 1024] pair psum)

# contraction chunks over DME=577: 4x128 + 65
CHUNKS = [(0, 128), (128, 128), (256, 128), (384, 128), (512, 65)]
# M chunks over 576 outputs: 4x128 + 64
MCHUNKS = [(0, 128), (128, 128), (256, 128), (384, 128), (512, 64)]

TYPE_A = dict(qs=[0, 256, 1536, 1792], skv=2048)
TYPE_B = dict(qs=[512, 768, 1024, 1280], skv=1536)

# head pairs for row-packed score matmuls: (head at array rows 0:64,
# head at rows 64:128); head 8 runs solo.
PAIRS = [(0, 1), (2, 3), (4, 5), (6, 7), (8,)]


def kv_of(h):
    return h // NKV


def build_program(qs_list, skv, reps=1, no_afsel=False, no_bcast=False, loop_reps=0):
    nc = bacc.Bacc("TRN2", target_bir_lowering=False, debug=False, num_devices=4)
    nkb = skv // 128
    nnb = skv // 512

    xt_d = nc.dram_tensor("xt", [DME, skv], F32R, kind="ExternalInput")
    wq_d = nc.dram_tensor("wq", [DME, DM], F32R, kind="ExternalInput")
    wk_d = nc.dram_tensor("wk", [DME, NKV * HD], F32R, kind="ExternalInput")
    wv_d = nc.dram_tensor("wv", [DME, NKV * HD], BF16, kind="ExternalInput")
    wo_d = nc.dram_tensor("wo", [DME, DM], F32R, kind="ExternalInput")
    ones_d = nc.dram_tensor("ones", [128, 9], F32R, kind="ExternalInput")
    out_d = nc.dram_tensor("out", [SQ, DM], F32, kind="ExternalOutput")

    # K/V projection coverage needed before superblocks 0 and 1 can run
    nkbs01 = max(qs_list[0], qs_list[1]) // 128 + 2
    p0_knb = (nkbs01 + 3) // 4          # K proj 512-col blocks in phase 0
    p0_vsb = nkbs01                     # V proj 128-col blocks in phase 0

    import contextlib
    with tile.TileContext(nc) as tc:
      for _rep in range(reps):
       with (tc.For_i(0, loop_reps, 1) if loop_reps else contextlib.nullcontext()):
        with (
            tc.tile_pool(name="const", bufs=1) as constp,
            tc.tile_pool(name="proj", bufs=1) as projp,
            tc.tile_pool(name="attn", bufs=1) as attnp,
            tc.tile_pool(name="psproj", bufs=2, space="PSUM") as psproj,
            tc.tile_pool(name="pssc", bufs=2, space="PSUM") as pssc,
            tc.tile_pool(name="psot", bufs=2, space="PSUM") as psot,
            tc.tile_pool(name="wtpool", bufs=5) as wtp,
            tc.tile_pool(name="small", bufs=4) as smallp,
            tc.tile_pool(name="outpool", bufs=4) as outp,
        ):
            # ---- small-weight loads first (unblock K/V projections) ----
            wk_sb, wv_sb = [], []
            for c, (r0, pc) in enumerate(CHUNKS):
                t = constp.tile([pc, NKV * HD], F32R, name=f"wk{c}", tag=f"wk{c}")
                nc.sync.dma_start(t, wk_d[r0:r0 + pc, :])
                wk_sb.append(t)
                t = constp.tile([pc, NKV * HD], BF16, name=f"wv{c}", tag=f"wv{c}")
                nc.sync.dma_start(t, wv_d[r0:r0 + pc, :])
                wv_sb.append(t)
            # ---- XT load, column-block-major so early key blocks land first
            xt_sb, xtbf_sb = [], []
            for c, (r0, pc) in enumerate(CHUNKS):
                t = constp.tile([pc, skv], F32R, name=f"xt{c}", tag=f"xt{c}")
                xt_sb.append(t)
                t = constp.tile([pc, skv], BF16, name=f"xtbf{c}", tag=f"xtbf{c}")
                xtbf_sb.append(t)
            xt_order = []
            for nb in (qs_list[0] // 512, 0, qs_list[2] // 512, *range(nnb)):
                if nb not in xt_order:
                    xt_order.append(nb)
            for nb in xt_order:
                sl = slice(nb * 512, (nb + 1) * 512)
                for c, (r0, pc) in enumerate(CHUNKS):
                    nc.sync.dma_start(xt_sb[c][:, sl], xt_d[r0:r0 + pc, sl])
                    nc.vector.tensor_copy(xtbf_sb[c][:, sl], xt_sb[c][:, sl])
            wq_sb, wo_sb = [], []
            for c, (r0, pc) in enumerate(CHUNKS):
                t = constp.tile([pc, DM], F32R, name=f"wq{c}", tag=f"wq{c}")
                nc.sync.dma_start(t, wq_d[r0:r0 + pc, :])
                wq_sb.append(t)
            for c, (r0, pc) in enumerate(CHUNKS):
                t = constp.tile([pc, DM], F32R, name=f"wo{c}", tag=f"wo{c}")
                nc.sync.dma_start(t, wo_d[r0:r0 + pc, :])
                wo_sb.append(t)

            # ---- constant causal mask tile [128, 2*SB] bf16:
            #   cols 0:SB   = keep where c >= p        (diag block nkbs-2)
            #   cols SB:2SB = keep where c >= p + 128  (diag block nkbs-1)
            mask2 = constp.tile([128, 2 * SB], BF16, name="mask2")
            nc.vector.memset(mask2, 1.0)
            for half, base in ((0, 0), (1, -128)):
                nc.gpsimd.affine_select(
                    out=mask2[:, half * SB:(half + 1) * SB],
                    in_=mask2[:, half * SB:(half + 1) * SB],
                    pattern=[[1, SB]],
                    compare_op=mybir.AluOpType.is_ge,
                    fill=0.0,
                    base=base,
                    channel_multiplier=-1,
                )
            # pair mask [M0|M1|M0|M1] view for one-shot masking of a pair tile
            maskp = constp.tile([128, 4 * SB], BF16, name="maskp")
            nc.vector.tensor_copy(maskp[:, 0:2 * SB], mask2)
            nc.vector.tensor_copy(maskp[:, 2 * SB:4 * SB], mask2)

            # KT + duplicated halves:
            #   kt0 [128, skv] = [kv0 ; kv1], kt1 [64, skv] = kv2
            #   dup0 [128, skv] = [kv1 ; kv0], dup1[64:128] = kv2
            kt0 = projp.tile([128, skv], F32R, name="kt0")
            kt1 = projp.tile([64, skv], F32R, name="kt1")
            dup0 = projp.tile([128, skv], F32R, name="dup0")
            dup1 = projp.tile([128, skv], F32R, name="dup1")
            vall = projp.tile([128, nkb, NKV, HD + 1], BF16, name="vall")
            nc.vector.memset(
                vall[:, :, :, HD:HD + 1].rearrange("p a b c -> p (a b c)"), 1.0
            )
            qt_sb = [
                projp.tile([mp, SQ], F32R, name=f"qt{c}", tag=f"qt{c}")
                for c, (m0, mp) in enumerate(MCHUNKS)
            ]

            def k_pass(nb):
                sl = slice(nb * 512, (nb + 1) * 512)
                ps = psproj.tile([128, 512], F32, name="psk", tag="psp")
                for k in range(5):
                    nc.tensor.matmul(
                        ps,
                        wk_sb[k][:, 0:128],
                        xt_sb[k][:, sl],
                        start=(k == 0), stop=(k == 4),
                    )
                nc.vector.tensor_copy(kt0[:, sl], ps)
                nc.vector.tensor_copy(dup0[0:64, sl], ps[64:128, :])
                nc.vector.tensor_copy(dup0[64:128, sl], ps[0:64, :])
                ps = psproj.tile([128, 512], F32, name="psk2", tag="psp")
                for k in range(5):
                    nc.tensor.matmul(
                        ps[:64, :],
                        wk_sb[k][:, 128:192],
                        xt_sb[k][:, sl],
                        start=(k == 0), stop=(k == 4),
                    )
                nc.vector.tensor_copy(kt1[:, sl], ps[0:64, :])
                nc.vector.tensor_copy(dup1[64:128, sl], ps[0:64, :])

            def q_pass(c, qb2):
                m0, mp = MCHUNKS[c]
                qg = qs_list[2 * qb2]
                ps = psproj.tile([128, 512], F32, name="psq", tag="psp")
                for k in range(5):
                    nc.tensor.matmul(
                        ps[:mp, :],
                        wq_sb[k][:, m0:m0 + mp],
                        xt_sb[k][:, qg:qg + 512],
                        start=(k == 0), stop=(k == 4),
                    )
                nc.vector.tensor_copy(
                    qt_sb[c][:, qb2 * 512:(qb2 + 1) * 512], ps[:mp, :]
                )

            def v_pass(sblk):
                psf = psproj.tile([128, 512], F32, name="psv", tag="psp")
                ps = psf[:, 0:NKV * HD]
                for k in range(5):
                    nc.tensor.matmul(
                        ps,
                        xtbf_sb[k][:, sblk * 128:(sblk + 1) * 128],
                        wv_sb[k],
                        start=(k == 0), stop=(k == 4),
                    )
                nc.vector.tensor_copy(
                    vall[:, sblk, :, 0:HD],
                    ps.rearrange("p (h d) -> p h d", h=NKV),
                )

            # phase 0: only what superblocks 0/1 need; rest becomes tasks
            # interleaved into the attention loop to fill PE idle time.
            # Q-proj chunk c is emitted just before superblock 0's pair c.
            for nb in range(p0_knb):
                k_pass(nb)
            for sblk in range(p0_vsb):
                v_pass(sblk)
            k_done = [nb < p0_knb for nb in range(nnb)]
            v_done = [sblk < p0_vsb for sblk in range(nkb)]
            # remaining passes ordered by the first kb block that needs them
            tasks = sorted(
                [("k", nb, nb * 4) for nb in range(p0_knb, nnb)]
                + [("v", sblk, sblk) for sblk in range(p0_vsb, nkb)],
                key=lambda t: (t[2], t[0]),
            )

            def emit_task(kind, arg):
                if kind == "k":
                    if not k_done[arg]:
                        k_done[arg] = True
                        k_pass(arg)
                else:
                    if not v_done[arg]:
                        v_done[arg] = True
                        v_pass(arg)

            def need_kb(kb):
                # forced deadline: K/V data for key block kb must exist before
                # the scores/attn@V matmuls that read it are emitted
                emit_task("k", kb // 4)
                emit_task("v", kb)
                while tasks and tasks[0][2] <= kb:
                    kind, arg, _ = tasks.pop(0)
                    emit_task(kind, arg)

            # KT slice for kv head at a given array-half (base 0 or 64)
            def kt_src(kv, base):
                if base == 0:
                    return [kt0[0:64, :], dup0[0:64, :], kt1[0:64, :]][kv]
                return [dup0[64:128, :], kt0[64:128, :], dup1[64:128, :]][kv]

            # attnT chunks [128, 1024] (+ ones row on chunk 4 for bo folding)
            at_sb = []
            for c in range(4):
                t = attnp.tile([128, SQ], F32R, name=f"at{c}", tag=f"at{c}")
                at_sb.append(t)
            t = attnp.tile([65, SQ], F32R, name="at4", tag="at4")
            nc.sync.dma_start(
                t[64:65, :],
                bass.AP(ones_d, 0, [[0, 1], [1, SQ]]),
            )
            at_sb.append(t)

            # ---- attention + per-superblock output projection ----
            # The attn@V matmuls for group g are emitted after the scores of
            # group g+1 (software pipelining): the PE queue is strict FIFO,
            # so emitting attn@V right after its own scores would head-block
            # the queue while the exp runs.
            def emit_attnv(st):
                # one accumulation chain per pair tile: PSUM groups are
                # bank-granular (start clears has_written for the whole bank),
                # so the two heads cannot run separate start/stop chains in
                # the shared [65, 512] bank.
                sb, pair, g, ot, wt, nkbs = st
                np_ = len(pair)
                ngrp = nkbs // G
                for i, h in enumerate(pair):
                    for j in range(G):
                        kb = g * G + j
                        nc.tensor.matmul(
                            ot[:, i * SB:(i + 1) * SB],
                            vall[:, kb, kv_of(h), :],
                            wt[:, (i * G + j) * SB:(i * G + j + 1) * SB],
                            start=(g == 0 and i == 0 and j == 0),
                            stop=(g == ngrp - 1 and i == np_ - 1 and j == G - 1),
                        )

            def emit_norm(st):
                sb, pair, g, ot, wt, nkbs = st
                np_ = len(pair)
                w = np_ * SB
                rec = smallp.tile([1, 512], F32, name="rec", tag="rec")
                nc.vector.reciprocal(rec[:, 0:w], ot[64:65, 0:w])
                bc = smallp.tile([64, 512], F32, name="bc", tag="bc")
                if no_bcast:
                    nc.vector.memset(bc, 1.0)
                else:
                    nc.gpsimd.partition_broadcast(bc[:, 0:w], rec[:, 0:w])
                for i, h in enumerate(pair):
                    r0 = (h % 2) * 64
                    nc.vector.tensor_mul(
                        at_sb[h // 2][r0:r0 + 64, sb * SB:(sb + 1) * SB],
                        ot[0:64, i * SB:(i + 1) * SB],
                        bc[:, i * SB:(i + 1) * SB],
                    )

            def flush(st):
                if st is None:
                    return
                emit_attnv(st)
                if st[2] == st[5] // G - 1:  # last group of its pair
                    emit_norm(st)

            pending = None
            for sb in range(NSB):
                qs = qs_list[sb]
                nkbs = qs // 128 + 2
                ngrp = nkbs // G
                for pi, pair in enumerate(PAIRS):
                    np_ = len(pair)
                    if sb == 0:
                        q_pass(pi, 0)
                        q_pass(pi, 1)
                    ot = psot.tile([65, 512], F32, name="ot", tag="ot")
                    qt_c = qt_sb[pair[0] // 2]
                    for g in range(ngrp):
                        need_kb(g * G + G - 1)
                        ps = pssc.tile([128, 2 * G * SB], F32, name="pssc",
                                       tag="pssc")
                        wt = wtp.tile([128, 2 * G * SB], BF16, name="wt",
                                      tag="wt")
                        for i, h in enumerate(pair):
                            base = (h % 2) * 64
                            lhs = kt_src(kv_of(h), base)
                            rhs = qt_c[base:base + 64, sb * SB:(sb + 1) * SB]
                            for j in range(G):
                                kb = g * G + j
                                nc.tensor.matmul(
                                    ps[:, (i * G + j) * SB:(i * G + j + 1) * SB],
                                    lhs[:, kb * 128:(kb + 1) * 128],
                                    rhs,
                                    start=True, stop=True,
                                    tile_position=(base, 0),
                                )
                        if g == 0 and sb > 0 and tasks:
                            kind, arg, _ = tasks.pop(0)
                            emit_task(kind, arg)
                        flush(pending)
                        w = np_ * G * SB
                        # exp(scores/8), psum -> sbuf bf16
                        nc.scalar.activation(
                            wt[:, 0:w], ps[:, 0:w],
                            func=mybir.ActivationFunctionType.Exp,
                            scale=0.125,
                        )
                        # causal mask on the last group's diagonal blocks
                        if g == ngrp - 1 and not no_afsel:
                            nc.vector.tensor_mul(
                                wt[:, 0:w], wt[:, 0:w], maskp[:, 0:w]
                            )
                        pending = (sb, pair, g, ot, wt, nkbs)
                flush(pending)
                pending = None
                # ---- output projection for this superblock's q rows ----
                for qb in range(2 * sb, 2 * sb + 2):
                    ott = outp.tile([128, DM], F32, name="outt", tag="outt")
                    for half in range(2):
                        pso = psot.tile([128, 288], F32, name="pso", tag="ot")
                        for c in range(5):
                            nc.tensor.matmul(
                                pso,
                                at_sb[c][:, qb * 128:(qb + 1) * 128],
                                wo_sb[c][:, half * 288:(half + 1) * 288],
                                start=(c == 0), stop=(c == 4),
                            )
                        nc.vector.tensor_copy(
                            ott[:, half * 288:(half + 1) * 288], pso
                        )
                    nc.sync.dma_start(out_d[qb * 128:(qb + 1) * 128, :], ott)

    nc.compile()
    return nc


_cache = {}


def _programs():
    if "A" not in _cache:
        _cache["A"] = build_program(TYPE_A["qs"], TYPE_A["skv"])
        _cache["B"] = build_program(TYPE_B["qs"], TYPE_B["skv"])
    return _cache["A"], _cache["B"]


def _make_runner(nc, devices):
    """Cached shard_map runner for `nc` pinned to an explicit device subset.

    Mirrors concourse.bass2jax.run_bass_via_pjrt's multi-core branch, but
    with a persistent jit and a caller-chosen device list so two programs
    can run concurrently on disjoint NeuronCore subsets.
    """
    import jax
    from jax.experimental.shard_map import shard_map
    from jax.sharding import Mesh, PartitionSpec
    from concourse import bass2jax, mybir as _mb

    bass2jax.install_neuronx_cc_hook()
    n_cores = len(devices)

    part_name = nc.partition_id_tensor.name if nc.partition_id_tensor else None
    in_names, out_names, out_avals = [], [], []
    for alloc in nc.m.functions[0].allocations:
        if not isinstance(alloc, mybir.MemoryLocationSet):
            continue
        name = alloc.memorylocations[0].name
        if alloc.kind == "ExternalInput":
            if name != part_name:
                in_names.append(name)
        elif alloc.kind == "ExternalOutput":
            out_names.append(name)
            out_avals.append(
                jax.core.ShapedArray(tuple(alloc.tensor_shape), _mb.dt.np(alloc.dtype))
            )
    n_params = len(in_names)
    n_outs = len(out_avals)
    all_names = in_names + out_names + ([part_name] if part_name else [])
    donate = tuple(range(n_params, n_params + n_outs))

    def _body(*args):
        args = list(args)
        if part_name:
            args.append(bass2jax.partition_id_tensor())
        outs = bass2jax._bass_exec_p.bind(
            *args,
            out_avals=tuple(out_avals),
            in_names=tuple(all_names),
            out_names=tuple(out_names),
            lowering_input_output_aliases=(),
            sim_require_finite=True,
            sim_require_nnan=True,
            nc=nc,
        )
        return tuple(outs)

    mesh = Mesh(np.asarray(devices), ("core",))
    in_specs = (PartitionSpec("core"),) * (n_params + n_outs)
    out_specs = (PartitionSpec("core"),) * n_outs
    sharded = jax.jit(
        shard_map(_body, mesh=mesh, in_specs=in_specs, out_specs=out_specs,
                  check_rep=False),
        donate_argnums=donate, keep_unused=True,
    )

    def run(in_maps, block=True):
        concat_in = [
            np.concatenate([np.asarray(m[name]) for m in in_maps], axis=0)
            for name in in_names
        ]
        zeros = [
            np.zeros((n_cores * a.shape[0], *a.shape[1:]), a.dtype) for a in out_avals
        ]
        out_arrs = sharded(*concat_in, *zeros)
        def collect():
            return [
                {name: np.asarray(out_arrs[i]).reshape(n_cores, *out_avals[i].shape)[c]
                 for i, name in enumerate(out_names)}
                for c in range(n_cores)
            ]
        return collect() if block else collect

    return run


def _runners():
    if "runA" not in _cache:
        import jax
        devs = jax.devices()
        ncA, ncB = _programs()
        _cache["runA"] = _make_runner(ncA, devs[0:4])
        _cache["runB"] = _make_runner(ncB, devs[4:8])
    return _cache["runA"], _cache["runB"]


def _host_inputs(inputs, Wq, bq, Wk, bk, Wv, bv, Wo, bo):
    x = np.asarray(inputs, dtype=np.float32)
    wq = np.vstack([np.asarray(Wq, np.float32), np.asarray(bq, np.float32)[None]])
    wk = np.vstack([np.asarray(Wk, np.float32), np.asarray(bk, np.float32)[None]])
    wv = np.vstack([np.asarray(Wv, np.float32), np.asarray(bv, np.float32)[None]]).astype(ml_dtypes.bfloat16)
    wo = np.vstack([np.asarray(Wo, np.float32), np.asarray(bo, np.float32)[None]])
    xts = []
    for b in range(B):
        xt = np.empty((DME, S), np.float32)
        xt[:DM] = x[b].T
        xt[DM] = 1.0
        xts.append(np.ascontiguousarray(xt))
    return xts, wq, wk, wv, wo


def kernel(inputs, Wq, bq, Wk, bk, Wv, bv, Wo, bo):
    xts, wq, wk, wv, wo = _host_inputs(inputs, Wq, bq, Wk, bk, Wv, bv, Wo, bo)
    ones = np.ones((128, 9), np.float32)
    maps_a = [dict(xt=xts[b], wq=wq, wk=wk, wv=wv, wo=wo, ones=ones)
              for b in range(B)]
    maps_b = [dict(xt=np.ascontiguousarray(xts[b][:, :TYPE_B["skv"]]), wq=wq,
                   wk=wk, wv=wv, wo=wo, ones=ones) for b in range(B)]
    try:
        run_a, run_b = _runners()
        col_a = run_a(maps_a, block=False)
        col_b = run_b(maps_b, block=False)
        res_a, res_b = col_a(), col_b()
    except Exception:
        res_a, res_b = _kernel_fallback(maps_a, maps_b)

    out = np.empty((B, S, DM), np.float32)
    for b in range(B):
        oa = res_a[b]["out"]
        ob = res_b[b]["out"]
        out[b, 0:512] = oa[0:512]
        out[b, 1536:2048] = oa[512:1024]
        out[b, 512:1536] = ob
    return out


def _kernel_fallback(maps_a, maps_b):
    ncA, ncB = _programs()
    res_a = run_bass_kernel_spmd(ncA, maps_a, core_ids=[0, 1, 2, 3]).results
    res_b = run_bass_kernel_spmd(ncB, maps_b, core_ids=[0, 1, 2, 3]).results
    return res_a, res_b


# revision 20
# speedup vs baseline: 3.7305x; 1.0029x over previous
"""GQA multi-head attention (B=4, S=2048, D=576, 9 Q heads / 3 KV heads,
causal) for 8 Trainium2 NeuronCores.

Sharding: 2 cores per batch item, split over the query dimension in
causally-balanced quarter pairs:
  type A core: q rows [0:512) + [1536:2048)   (kb counts 2,4,14,16 per 256-sb)
  type B core: q rows [512:1536)              (kb counts 6,8,10,12)
Each core redundantly computes K/V projections for the keys it needs.
Two compiled programs (A and B), 4 cores each.

Layout strategy (everything "transposed", d_model on partitions):
  XT_ext [577, Skv]  (row 576 = ones, folds biases into projections)
  QT     [576, 1024] (head h at chunk h//2, partition (h%2)*64)
  KT     [192, Skv] + duplicated halves for row-packed score matmuls
  V      [Skv, 3, 65] bf16 (65th col = ones -> softmax denominator)
  scores^T [k, q] psum shared per head pair -> one exp (ACT, bf16 out)
  -> causal mask via one DVE multiply with a constant 0/1 tile (diagonal
  blocks only) -> attn@V psum [65, 512] per pair accumulated over k
  normalize: DVE reciprocal of row 64, gpsimd partition_broadcast, DVE mul
  out-proj: per superblock; lhsT = attnT chunks (ones row folds bo)
K/V projection passes not needed by the first two superblocks are
interleaved into the attention loop to fill tensor-engine idle time.
Score/Q/K/out matmuls fp32r (full rate at free dim >= 256); V projection
and softmax weights bf16 (fp32r is quarter-rate below free dim 256).
"""

import numpy as np
import ml_dtypes

import concourse.bass as bass
import concourse.bacc as bacc
import concourse.tile as tile
from concourse import mybir
from concourse.bass_utils import run_bass_kernel_spmd

F32 = mybir.dt.float32
F32R = mybir.dt.float32r
BF16 = mybir.dt.bfloat16

B, S, DM = 4, 2048, 576
DME = DM + 1          # + ones row for bias folding
HD = 64               # head dim
NH = 9                # query heads
NKV = 3               # kv heads
SB = 256              # q superblock (free dim of score matmuls)
NSB = 4               # q superblocks per core (1024 q rows)
SQ = SB * NSB
G = 2                 # kb-blocks per head per exp batch ([128, 1024] pair psum)

# contraction chunks over DME=577: 4x128 + 65
CHUNKS = [(0, 128), (128, 128), (256, 128), (384, 128), (512, 65)]
# M chunks over 576 outputs: 4x128 + 64
MCHUNKS = [(0, 128), (128, 128), (256, 128), (384, 128), (512, 64)]

TYPE_A = dict(qs=[0, 256, 1536, 1792], skv=2048)
TYPE_B = dict(qs=[512, 768, 1024, 1280], skv=1536)

# head pairs for row-packed score matmuls: (head at array rows 0:64,
# head at rows 64:128); head 8 runs solo.
PAIRS = [(0, 1), (2, 3), (4, 5), (6, 7), (8,)]


def kv_of(h):
    return h // NKV


def build_program(qs_list, skv, reps=1, no_afsel=False, no_bcast=False, loop_reps=0):
    nc = bacc.Bacc("TRN2", target_bir_lowering=False, debug=False, num_devices=4)
    nkb = skv // 128
    nnb = skv // 512

    xt_d = nc.dram_tensor("xt", [DME, skv], BF16, kind="ExternalInput")
    wq_d = nc.dram_tensor("wq", [DME, DM], BF16, kind="ExternalInput")
    wk_d = nc.dram_tensor("wk", [DME, NKV * HD], BF16, kind="ExternalInput")
    wv_d = nc.dram_tensor("wv", [DME, NKV * HD], BF16, kind="ExternalInput")
    wo_d = nc.dram_tensor("wo", [DME, DM], BF16, kind="ExternalInput")
    out_d = nc.dram_tensor("out", [SQ, DM], BF16, kind="ExternalOutput")

    # K/V projection coverage needed before superblocks 0 and 1 can run
    nkbs01 = max(qs_list[0], qs_list[1]) // 128 + 2
    p0_knb = (nkbs01 + 3) // 4          # K proj 512-col blocks in phase 0
    p0_vsb = nkbs01                     # V proj 128-col blocks in phase 0

    import contextlib
    with tile.TileContext(nc) as tc:
      for _rep in range(reps):
       with (tc.For_i(0, loop_reps, 1) if loop_reps else contextlib.nullcontext()):
        with (
            tc.tile_pool(name="const", bufs=1) as constp,
            tc.tile_pool(name="proj", bufs=1) as projp,
            tc.tile_pool(name="attn", bufs=1) as attnp,
            tc.tile_pool(name="psproj", bufs=2, space="PSUM") as psproj,
            tc.tile_pool(name="pssc", bufs=2, space="PSUM") as pssc,
            tc.tile_pool(name="psot", bufs=2, space="PSUM") as psot,
            tc.tile_pool(name="wtpool", bufs=5) as wtp,
            tc.tile_pool(name="small", bufs=4) as smallp,
            tc.tile_pool(name="outpool", bufs=4) as outp,
        ):
            # ---- small-weight loads first (unblock K/V projections) ----
            wk_sb, wv_sb = [], []
            for c, (r0, pc) in enumerate(CHUNKS):
                t = constp.tile([pc, NKV * HD], BF16, name=f"wk{c}", tag=f"wk{c}")
                nc.sync.dma_start(t, wk_d[r0:r0 + pc, :])
                wk_sb.append(t)
                t = constp.tile([pc, NKV * HD], BF16, name=f"wv{c}", tag=f"wv{c}")
                nc.sync.dma_start(t, wv_d[r0:r0 + pc, :])
                wv_sb.append(t)
            # ---- XT load, column-block-major so early key blocks land first
            xt_sb = []
            for c, (r0, pc) in enumerate(CHUNKS):
                t = constp.tile([pc, skv], BF16, name=f"xt{c}", tag=f"xt{c}")
                xt_sb.append(t)
            xt_order = []
            for nb in (qs_list[0] // 512, 0, qs_list[2] // 512, *range(nnb)):
                if nb not in xt_order:
                    xt_order.append(nb)
            for nb in xt_order:
                sl = slice(nb * 512, (nb + 1) * 512)
                for c, (r0, pc) in enumerate(CHUNKS):
                    nc.sync.dma_start(xt_sb[c][:, sl], xt_d[r0:r0 + pc, sl])
            wq_sb, wo_sb = [], []
            for c, (r0, pc) in enumerate(CHUNKS):
                t = constp.tile([pc, DM], BF16, name=f"wq{c}", tag=f"wq{c}")
                nc.sync.dma_start(t, wq_d[r0:r0 + pc, :])
                wq_sb.append(t)
            for c, (r0, pc) in enumerate(CHUNKS):
                t = constp.tile([pc, DM], BF16, name=f"wo{c}", tag=f"wo{c}")
                nc.sync.dma_start(t, wo_d[r0:r0 + pc, :])
                wo_sb.append(t)

            # ---- constant causal mask tile [128, 2*SB] bf16:
            #   cols 0:SB   = keep where c >= p        (diag block nkbs-2)
            #   cols SB:2SB = keep where c >= p + 128  (diag block nkbs-1)
            mask2 = constp.tile([128, 2 * SB], BF16, name="mask2")
            nc.vector.memset(mask2, 1.0)
            for half, base in ((0, 0), (1, -128)):
                nc.gpsimd.affine_select(
                    out=mask2[:, half * SB:(half + 1) * SB],
                    in_=mask2[:, half * SB:(half + 1) * SB],
                    pattern=[[1, SB]],
                    compare_op=mybir.AluOpType.is_ge,
                    fill=0.0,
                    base=base,
                    channel_multiplier=-1,
                )
            # pair mask [M0|M1|M0|M1] view for one-shot masking of a pair tile
            maskp = constp.tile([128, 4 * SB], BF16, name="maskp")
            nc.vector.tensor_copy(maskp[:, 0:2 * SB], mask2)
            nc.vector.tensor_copy(maskp[:, 2 * SB:4 * SB], mask2)

            # KT + duplicated halves:
            #   kt0 [128, skv] = [kv0 ; kv1], kt1 [64, skv] = kv2
            #   dup0 [128, skv] = [kv1 ; kv0], dup1[64:128] = kv2
            kt0 = projp.tile([128, skv], F32R, name="kt0")
            kt1 = projp.tile([64, skv], F32R, name="kt1")
            dup0 = projp.tile([128, skv], F32R, name="dup0")
            dup1 = projp.tile([128, skv], F32R, name="dup1")
            vall = projp.tile([128, nkb, NKV, HD + 1], BF16, name="vall")
            nc.vector.memset(
                vall[:, :, :, HD:HD + 1].rearrange("p a b c -> p (a b c)"), 1.0
            )
            qt_sb = [
                projp.tile([mp, SQ], F32R, name=f"qt{c}", tag=f"qt{c}")
                for c, (m0, mp) in enumerate(MCHUNKS)
            ]

            def _evac(on_act, dst, src):
                # phase-0 evacuations ride the then-idle ACT engine; the
                # interleaved task-phase ones use DVE (ACT is exp-bound then)
                if on_act:
                    nc.scalar.activation(
                        dst, src, func=mybir.ActivationFunctionType.Copy
                    )
                else:
                    nc.vector.tensor_copy(dst, src)

            def k_pass(nb, on_act=False):
                sl = slice(nb * 512, (nb + 1) * 512)
                ps = psproj.tile([128, 512], F32, name="psk", tag="psp")
                for k in range(5):
                    nc.tensor.matmul(
                        ps,
                        wk_sb[k][:, 0:128],
                        xt_sb[k][:, sl],
                        start=(k == 0), stop=(k == 4),
                    )
                _evac(on_act, kt0[:, sl], ps)
                _evac(on_act, dup0[0:64, sl], ps[64:128, :])
                _evac(on_act, dup0[64:128, sl], ps[0:64, :])
                ps = psproj.tile([128, 512], F32, name="psk2", tag="psp")
                for k in range(5):
                    nc.tensor.matmul(
                        ps[:64, :],
                        wk_sb[k][:, 128:192],
                        xt_sb[k][:, sl],
                        start=(k == 0), stop=(k == 4),
                    )
                _evac(on_act, kt1[:, sl], ps[0:64, :])
                _evac(on_act, dup1[64:128, sl], ps[0:64, :])

            def q_pass(c, qb2, on_act=False):
                m0, mp = MCHUNKS[c]
                qg = qs_list[2 * qb2]
                ps = psproj.tile([128, 512], F32, name="psq", tag="psp")
                for k in range(5):
                    nc.tensor.matmul(
                        ps[:mp, :],
                        wq_sb[k][:, m0:m0 + mp],
                        xt_sb[k][:, qg:qg + 512],
                        start=(k == 0), stop=(k == 4),
                    )
                _evac(on_act, qt_sb[c][:, qb2 * 512:(qb2 + 1) * 512], ps[:mp, :])

            def v_pass(sblk, on_act=False):
                psf = psproj.tile([128, 512], F32, name="psv", tag="psp")
                ps = psf[:, 0:NKV * HD]
                for k in range(5):
                    nc.tensor.matmul(
                        ps,
                        xt_sb[k][:, sblk * 128:(sblk + 1) * 128],
                        wv_sb[k],
                        start=(k == 0), stop=(k == 4),
                    )
                _evac(
                    on_act,
                    vall[:, sblk, :, 0:HD],
                    ps.rearrange("p (h d) -> p h d", h=NKV),
                )

            # phase 0: only what superblocks 0/1 need; rest becomes tasks
            # interleaved into the attention loop to fill PE idle time.
            # Q-proj chunk c is emitted just before superblock 0's pair c.
            for nb in range(p0_knb):
                k_pass(nb, on_act=True)
            for sblk in range(p0_vsb):
                v_pass(sblk, on_act=True)
            k_done = [nb < p0_knb for nb in range(nnb)]
            v_done = [sblk < p0_vsb for sblk in range(nkb)]
            # remaining passes ordered by the first kb block that needs them
            tasks = sorted(
                [("k", nb, nb * 4) for nb in range(p0_knb, nnb)]
                + [("v", sblk, sblk) for sblk in range(p0_vsb, nkb)],
                key=lambda t: (t[2], t[0]),
            )

            def emit_task(kind, arg):
                if kind == "k":
                    if not k_done[arg]:
                        k_done[arg] = True
                        k_pass(arg)
                else:
                    if not v_done[arg]:
                        v_done[arg] = True
                        v_pass(arg)

            def need_kb(kb):
                # forced deadline: K/V data for key block kb must exist before
                # the scores/attn@V matmuls that read it are emitted
                emit_task("k", kb // 4)
                emit_task("v", kb)
                while tasks and tasks[0][2] <= kb:
                    kind, arg, _ = tasks.pop(0)
                    emit_task(kind, arg)

            # KT slice for kv head at a given array-half (base 0 or 64)
            def kt_src(kv, base):
                if base == 0:
                    return [kt0[0:64, :], dup0[0:64, :], kt1[0:64, :]][kv]
                return [dup0[64:128, :], kt0[64:128, :], dup1[64:128, :]][kv]

            # attnT chunks [128, 1024] (+ ones row on chunk 4 for bo folding)
            at_sb = []
            for c in range(4):
                t = attnp.tile([128, SQ], BF16, name=f"at{c}", tag=f"at{c}")
                at_sb.append(t)
            t = attnp.tile([65, SQ], BF16, name="at4", tag="at4")
            nc.vector.memset(t[64:65, :], 1.0)
            at_sb.append(t)

            # ---- attention + per-superblock output projection ----
            # The attn@V matmuls for group g are emitted after the scores of
            # group g+1 (software pipelining): the PE queue is strict FIFO,
            # so emitting attn@V right after its own scores would head-block
            # the queue while the exp runs.
            def emit_attnv(st):
                # one accumulation chain per pair tile: PSUM groups are
                # bank-granular (start clears has_written for the whole bank),
                # so the two heads cannot run separate start/stop chains in
                # the shared [65, 512] bank.
                sb, pair, g, ot, wt, nkbs = st
                np_ = len(pair)
                ngrp = nkbs // G
                for i, h in enumerate(pair):
                    for j in range(G):
                        kb = g * G + j
                        nc.tensor.matmul(
                            ot[:, i * SB:(i + 1) * SB],
                            vall[:, kb, kv_of(h), :],
                            wt[:, (i * G + j) * SB:(i * G + j + 1) * SB],
                            start=(g == 0 and i == 0 and j == 0),
                            stop=(g == ngrp - 1 and i == np_ - 1 and j == G - 1),
                        )

            def emit_norm(st):
                sb, pair, g, ot, wt, nkbs = st
                np_ = len(pair)
                w = np_ * SB
                rec = smallp.tile([1, 512], F32, name="rec", tag="rec")
                nc.vector.reciprocal(rec[:, 0:w], ot[64:65, 0:w])
                bc = smallp.tile([64, 512], F32, name="bc", tag="bc")
                if no_bcast:
                    nc.vector.memset(bc, 1.0)
                else:
                    nc.gpsimd.partition_broadcast(bc[:, 0:w], rec[:, 0:w])
                for i, h in enumerate(pair):
                    r0 = (h % 2) * 64
                    nc.vector.tensor_mul(
                        at_sb[h // 2][r0:r0 + 64, sb * SB:(sb + 1) * SB],
                        ot[0:64, i * SB:(i + 1) * SB],
                        bc[:, i * SB:(i + 1) * SB],
                    )

            def flush(st):
                if st is None:
                    return
                emit_attnv(st)
                if st[2] == st[5] // G - 1:  # last group of its pair
                    emit_norm(st)

            pending = None
            for sb in range(NSB):
                qs = qs_list[sb]
                nkbs = qs // 128 + 2
                ngrp = nkbs // G
                for pi, pair in enumerate(PAIRS):
                    np_ = len(pair)
                    if sb == 0:
                        q_pass(pi, 0, on_act=True)
                        q_pass(pi, 1, on_act=True)
                    ot = psot.tile([65, 512], F32, name="ot", tag="ot")
                    qt_c = qt_sb[pair[0] // 2]
                    for g in range(ngrp):
                        need_kb(g * G + G - 1)
                        ps = pssc.tile([128, 2 * G * SB], F32, name="pssc",
                                       tag="pssc")
                        wt = wtp.tile([128, 2 * G * SB], BF16, name="wt",
                                      tag="wt")
                        for i, h in enumerate(pair):
                            base = (h % 2) * 64
                            lhs = kt_src(kv_of(h), base)
                            rhs = qt_c[base:base + 64, sb * SB:(sb + 1) * SB]
                            for j in range(G):
                                kb = g * G + j
                                nc.tensor.matmul(
                                    ps[:, (i * G + j) * SB:(i * G + j + 1) * SB],
                                    lhs[:, kb * 128:(kb + 1) * 128],
                                    rhs,
                                    start=True, stop=True,
                                    tile_position=(base, 0),
                                )
                        if g == 0 and sb > 0 and tasks:
                            kind, arg, _ = tasks.pop(0)
                            emit_task(kind, arg)
                        flush(pending)
                        w = np_ * G * SB
                        # exp(scores/8), psum -> sbuf bf16
                        nc.scalar.activation(
                            wt[:, 0:w], ps[:, 0:w],
                            func=mybir.ActivationFunctionType.Exp,
                            scale=0.125,
                        )
                        # causal mask on the last group's diagonal blocks
                        if g == ngrp - 1 and not no_afsel:
                            nc.vector.tensor_mul(
                                wt[:, 0:w], wt[:, 0:w], maskp[:, 0:w]
                            )
                        pending = (sb, pair, g, ot, wt, nkbs)
                flush(pending)
                pending = None
                # ---- output projection for this superblock's q rows ----
                for qb in range(2 * sb, 2 * sb + 2):
                    ott = outp.tile([128, DM], BF16, name="outt", tag="outt")
                    for half in range(2):
                        pso = psot.tile([128, 288], F32, name="pso", tag="ot")
                        for c in range(5):
                            nc.tensor.matmul(
                                pso,
                                at_sb[c][:, qb * 128:(qb + 1) * 128],
                                wo_sb[c][:, half * 288:(half + 1) * 288],
                                start=(c == 0), stop=(c == 4),
                            )
                        nc.vector.tensor_copy(
                            ott[:, half * 288:(half + 1) * 288], pso
                        )
                    nc.sync.dma_start(out_d[qb * 128:(qb + 1) * 128, :], ott)

    nc.compile()
    return nc


_cache = {}


def _programs():
    if "A" not in _cache:
        _cache["A"] = build_program(TYPE_A["qs"], TYPE_A["skv"])
        _cache["B"] = build_program(TYPE_B["qs"], TYPE_B["skv"])
    return _cache["A"], _cache["B"]


def _make_runner(nc, devices):
    """Cached shard_map runner for `nc` pinned to an explicit device subset.

    Mirrors concourse.bass2jax.run_bass_via_pjrt's multi-core branch, but
    with a persistent jit and a caller-chosen device list so two programs
    can run concurrently on disjoint NeuronCore subsets.
    """
    import jax
    from jax.experimental.shard_map import shard_map
    from jax.sharding import Mesh, PartitionSpec
    from concourse import bass2jax, mybir as _mb

    bass2jax.install_neuronx_cc_hook()
    n_cores = len(devices)

    part_name = nc.partition_id_tensor.name if nc.partition_id_tensor else None
    in_names, out_names, out_avals = [], [], []
    for alloc in nc.m.functions[0].allocations:
        if not isinstance(alloc, mybir.MemoryLocationSet):
            continue
        name = alloc.memorylocations[0].name
        if alloc.kind == "ExternalInput":
            if name != part_name:
                in_names.append(name)
        elif alloc.kind == "ExternalOutput":
            out_names.append(name)
            out_avals.append(
                jax.core.ShapedArray(tuple(alloc.tensor_shape), _mb.dt.np(alloc.dtype))
            )
    n_params = len(in_names)
    n_outs = len(out_avals)
    all_names = in_names + out_names + ([part_name] if part_name else [])
    donate = tuple(range(n_params, n_params + n_outs))

    def _body(*args):
        args = list(args)
        if part_name:
            args.append(bass2jax.partition_id_tensor())
        outs = bass2jax._bass_exec_p.bind(
            *args,
            out_avals=tuple(out_avals),
            in_names=tuple(all_names),
            out_names=tuple(out_names),
            lowering_input_output_aliases=(),
            sim_require_finite=True,
            sim_require_nnan=True,
            nc=nc,
        )
        return tuple(outs)

    mesh = Mesh(np.asarray(devices), ("core",))
    in_specs = (PartitionSpec("core"),) * (n_params + n_outs)
    out_specs = (PartitionSpec("core"),) * n_outs
    sharded = jax.jit(
        shard_map(_body, mesh=mesh, in_specs=in_specs, out_specs=out_specs,
                  check_rep=False),
        donate_argnums=donate, keep_unused=True,
    )

    def run(in_maps, block=True):
        concat_in = [
            np.concatenate([np.asarray(m[name]) for m in in_maps], axis=0)
            for name in in_names
        ]
        zeros = [
            np.zeros((n_cores * a.shape[0], *a.shape[1:]), a.dtype) for a in out_avals
        ]
        out_arrs = sharded(*concat_in, *zeros)
        def collect():
            return [
                {name: np.asarray(out_arrs[i]).reshape(n_cores, *out_avals[i].shape)[c]
                 for i, name in enumerate(out_names)}
                for c in range(n_cores)
            ]
        return collect() if block else collect

    return run


def _runners():
    if "runA" not in _cache:
        import jax
        devs = jax.devices()
        ncA, ncB = _programs()
        _cache["runA"] = _make_runner(ncA, devs[0:4])
        _cache["runB"] = _make_runner(ncB, devs[4:8])
    return _cache["runA"], _cache["runB"]


def _host_inputs(inputs, Wq, bq, Wk, bk, Wv, bv, Wo, bo):
    bf = ml_dtypes.bfloat16
    x = np.asarray(inputs, dtype=np.float32)
    wq = np.vstack([np.asarray(Wq, np.float32), np.asarray(bq, np.float32)[None]]).astype(bf)
    wk = np.vstack([np.asarray(Wk, np.float32), np.asarray(bk, np.float32)[None]]).astype(bf)
    wv = np.vstack([np.asarray(Wv, np.float32), np.asarray(bv, np.float32)[None]]).astype(bf)
    wo = np.vstack([np.asarray(Wo, np.float32), np.asarray(bo, np.float32)[None]]).astype(bf)
    xts = []
    for b in range(B):
        xt = np.empty((DME, S), bf)
        xt[:DM] = x[b].T.astype(bf)
        xt[DM] = 1.0
        xts.append(np.ascontiguousarray(xt))
    return xts, wq, wk, wv, wo


def kernel(inputs, Wq, bq, Wk, bk, Wv, bv, Wo, bo):
    xts, wq, wk, wv, wo = _host_inputs(inputs, Wq, bq, Wk, bk, Wv, bv, Wo, bo)
    ones = np.ones((128, 9), np.float32)
    maps_a = [dict(xt=xts[b], wq=wq, wk=wk, wv=wv, wo=wo, ones=ones)
              for b in range(B)]
    maps_b = [dict(xt=np.ascontiguousarray(xts[b][:, :TYPE_B["skv"]]), wq=wq,
                   wk=wk, wv=wv, wo=wo, ones=ones) for b in range(B)]
    try:
        run_a, run_b = _runners()
        col_a = run_a(maps_a, block=False)
        col_b = run_b(maps_b, block=False)
        res_a, res_b = col_a(), col_b()
    except Exception:
        res_a, res_b = _kernel_fallback(maps_a, maps_b)

    out = np.empty((B, S, DM), np.float32)
    for b in range(B):
        oa = np.asarray(res_a[b]["out"], np.float32)
        ob = np.asarray(res_b[b]["out"], np.float32)
        out[b, 0:512] = oa[0:512]
        out[b, 1536:2048] = oa[512:1024]
        out[b, 512:1536] = ob
    return out


def _kernel_fallback(maps_a, maps_b):
    ncA, ncB = _programs()
    res_a = run_bass_kernel_spmd(ncA, maps_a, core_ids=[0, 1, 2, 3]).results
    res_b = run_bass_kernel_spmd(ncB, maps_b, core_ids=[0, 1, 2, 3]).results
    return res_a, res_b


# revision 28
# speedup vs baseline: 4.7102x; 1.2626x over previous
"""GQA multi-head attention (B=4, S=2048, D=576, 9 Q heads / 3 KV heads,
causal) for 8 Trainium2 NeuronCores.

Sharding: 2 cores per batch item, split over the query dimension in
causally-balanced quarter pairs:
  type A core: q rows [0:512) + [1536:2048)   (kb counts 2,4,14,16 per 256-sb)
  type B core: q rows [512:1536)              (kb counts 6,8,10,12)
Each core redundantly computes K/V projections for the keys it needs.
Two compiled programs (A and B), 4 cores each.

Layout strategy (everything "transposed", d_model on partitions):
  XT_ext [577, Skv]  (row 576 = ones, folds biases into projections)
  QT     [576, 1024] (head h at chunk h//2, partition (h%2)*64)
  KT     [192, Skv] + duplicated halves for row-packed score matmuls
  V      [Skv, 3, 65] bf16 (65th col = ones -> softmax denominator)
  scores^T [k, q] psum shared per head pair -> one exp (ACT, bf16 out)
  -> causal mask via one DVE multiply with a constant 0/1 tile (diagonal
  blocks only) -> attn@V psum [65, 512] per pair accumulated over k
  normalize: DVE reciprocal of row 64, gpsimd partition_broadcast, DVE mul
  out-proj: per superblock; lhsT = attnT chunks (ones row folds bo)
K/V projection passes not needed by the first two superblocks are
interleaved into the attention loop to fill tensor-engine idle time.
Score/Q/K/out matmuls fp32r (full rate at free dim >= 256); V projection
and softmax weights bf16 (fp32r is quarter-rate below free dim 256).
"""

import numpy as np
import ml_dtypes

import concourse.bass as bass
import concourse.bacc as bacc
import concourse.tile as tile
from concourse import mybir
from concourse.bass_utils import run_bass_kernel_spmd

F32 = mybir.dt.float32
F32R = mybir.dt.float32r
BF16 = mybir.dt.bfloat16

B, S, DM = 4, 2048, 576
DME = DM + 1          # + ones row for bias folding
HD = 64               # head dim
NH = 9                # query heads
NKV = 3               # kv heads
SB = 256              # q superblock (free dim of score matmuls)
NSB = 4               # q superblocks per core (1024 q rows)
SQ = SB * NSB
G = 2                 # kb-blocks per head per exp batch ([128, 1024] pair psum)

# contraction chunks over DME=577: 4x128 + 65
CHUNKS = [(0, 128), (128, 128), (256, 128), (384, 128), (512, 65)]
# M chunks over 576 outputs: 4x128 + 64
MCHUNKS = [(0, 128), (128, 128), (256, 128), (384, 128), (512, 64)]

TYPE_A = dict(qs=[0, 256, 1536, 1792], skv=2048)
TYPE_B = dict(qs=[512, 768, 1024, 1280], skv=1536)

# head pairs for row-packed score matmuls: (head at array rows 0:64,
# head at rows 64:128); head 8 runs solo.
PAIRS = [(0, 1), (2, 3), (4, 5), (6, 7), (8,)]


def kv_of(h):
    return h // NKV


def build_program(qs_list, skv, reps=1, no_afsel=False, no_bcast=False, loop_reps=0):
    nc = bacc.Bacc("TRN2", target_bir_lowering=False, debug=False, num_devices=4)
    nkb = skv // 128
    nnb = skv // 512

    xt_d = nc.dram_tensor("xt", [DME, skv], BF16, kind="ExternalInput")
    wq_d = nc.dram_tensor("wq", [DME, DM], BF16, kind="ExternalInput")
    wk_d = nc.dram_tensor("wk", [DME, NKV * HD], BF16, kind="ExternalInput")
    wv_d = nc.dram_tensor("wv", [DME, NKV * HD], BF16, kind="ExternalInput")
    wo_d = nc.dram_tensor("wo", [DME, DM], BF16, kind="ExternalInput")
    out_d = nc.dram_tensor("out", [SQ, DM], BF16, kind="ExternalOutput")

    # superblock processing order: second-smallest first (cheap rampup while
    # DMA lands), largest next (ACT-bound steady state starts early, K/V
    # tasks drain into its pipeline), smallest last (short tail)
    sb_order = [1, 3, 2, 0]
    # K/V projection coverage needed before the first processed superblock
    nkbs0 = qs_list[sb_order[0]] // 128 + 2
    p0_knb = (nkbs0 + 3) // 4           # K proj 512-col blocks in phase 0
    p0_vsb = nkbs0                      # V proj 128-col blocks in phase 0

    import contextlib
    with tile.TileContext(nc) as tc:
      for _rep in range(reps):
       with (tc.For_i(0, loop_reps, 1) if loop_reps else contextlib.nullcontext()):
        with (
            tc.tile_pool(name="const", bufs=1) as constp,
            tc.tile_pool(name="proj", bufs=1) as projp,
            tc.tile_pool(name="attn", bufs=1) as attnp,
            tc.tile_pool(name="psproj", bufs=2, space="PSUM") as psproj,
            tc.tile_pool(name="pssc", bufs=2, space="PSUM") as pssc,
            tc.tile_pool(name="psot", bufs=2, space="PSUM") as psot,
            tc.tile_pool(name="wtpool", bufs=5) as wtp,
            tc.tile_pool(name="small", bufs=4) as smallp,
            tc.tile_pool(name="outpool", bufs=4) as outp,
        ):
            # ---- DMA issue order matches first consumers: wk, first xt
            # block (K proj), wv (V proj), wq (Q proj), remaining xt, wo
            xt_order = []
            for nb in (qs_list[0] // 512, 0, qs_list[2] // 512, *range(nnb)):
                if nb not in xt_order:
                    xt_order.append(nb)
            wk_sb = []
            for c, (r0, pc) in enumerate(CHUNKS):
                t = constp.tile([pc, NKV * HD], BF16, name=f"wk{c}", tag=f"wk{c}")
                nc.sync.dma_start(t, wk_d[r0:r0 + pc, :])
                wk_sb.append(t)
            xt_sb = []
            for c, (r0, pc) in enumerate(CHUNKS):
                t = constp.tile([pc, skv], BF16, name=f"xt{c}", tag=f"xt{c}")
                xt_sb.append(t)

            def xt_dma(nb):
                sl = slice(nb * 512, (nb + 1) * 512)
                for c, (r0, pc) in enumerate(CHUNKS):
                    nc.sync.dma_start(xt_sb[c][:, sl], xt_d[r0:r0 + pc, sl])

            xt_dma(xt_order[0])
            wv_sb = []
            for c, (r0, pc) in enumerate(CHUNKS):
                t = constp.tile([pc, NKV * HD], BF16, name=f"wv{c}", tag=f"wv{c}")
                nc.sync.dma_start(t, wv_d[r0:r0 + pc, :])
                wv_sb.append(t)
            wq_sb, wo_sb = [], []
            for c, (r0, pc) in enumerate(CHUNKS):
                t = constp.tile([pc, DM], BF16, name=f"wq{c}", tag=f"wq{c}")
                nc.sync.dma_start(t, wq_d[r0:r0 + pc, :])
                wq_sb.append(t)
            for nb in xt_order[1:]:
                xt_dma(nb)
            for c, (r0, pc) in enumerate(CHUNKS):
                t = constp.tile([pc, DM], BF16, name=f"wo{c}", tag=f"wo{c}")
                nc.sync.dma_start(t, wo_d[r0:r0 + pc, :])
                wo_sb.append(t)

            # ---- constant causal mask tile [128, 2*SB] bf16:
            #   cols 0:SB   = keep where c >= p        (diag block nkbs-2)
            #   cols SB:2SB = keep where c >= p + 128  (diag block nkbs-1)
            mask2 = constp.tile([128, 2 * SB], BF16, name="mask2")
            nc.vector.memset(mask2, 1.0)
            for half, base in ((0, 0), (1, -128)):
                nc.gpsimd.affine_select(
                    out=mask2[:, half * SB:(half + 1) * SB],
                    in_=mask2[:, half * SB:(half + 1) * SB],
                    pattern=[[1, SB]],
                    compare_op=mybir.AluOpType.is_ge,
                    fill=0.0,
                    base=base,
                    channel_multiplier=-1,
                )
            # pair mask [M0|M1|M0|M1] view for one-shot masking of a pair tile
            maskp = constp.tile([128, 4 * SB], BF16, name="maskp")
            nc.vector.tensor_copy(maskp[:, 0:2 * SB], mask2)
            nc.vector.tensor_copy(maskp[:, 2 * SB:4 * SB], mask2)

            # KT + duplicated halves:
            #   kt0 [128, skv] = [kv0 ; kv1], kt1 [64, skv] = kv2
            #   dup0 [128, skv] = [kv1 ; kv0], dup1[64:128] = kv2
            kt0 = projp.tile([128, skv], F32R, name="kt0")
            kt1 = projp.tile([64, skv], F32R, name="kt1")
            dup0 = projp.tile([128, skv], F32R, name="dup0")
            dup1 = projp.tile([128, skv], F32R, name="dup1")
            vall = projp.tile([128, nkb, NKV, HD + 1], BF16, name="vall")
            nc.vector.memset(
                vall[:, :, :, HD:HD + 1].rearrange("p a b c -> p (a b c)"), 1.0
            )
            qt_sb = [
                projp.tile([mp, SQ], F32R, name=f"qt{c}", tag=f"qt{c}")
                for c, (m0, mp) in enumerate(MCHUNKS)
            ]

            def _evac(on_act, dst, src):
                # phase-0 evacuations ride the then-idle ACT engine; the
                # interleaved task-phase ones use DVE (ACT is exp-bound then)
                if on_act:
                    nc.scalar.activation(
                        dst, src, func=mybir.ActivationFunctionType.Copy
                    )
                else:
                    nc.vector.tensor_copy(dst, src)

            def k_pass(nb, on_act=False):
                sl = slice(nb * 512, (nb + 1) * 512)
                ps = psproj.tile([128, 512], F32, name="psk", tag="psp")
                for k in range(5):
                    nc.tensor.matmul(
                        ps,
                        wk_sb[k][:, 0:128],
                        xt_sb[k][:, sl],
                        start=(k == 0), stop=(k == 4),
                    )
                _evac(on_act, kt0[:, sl], ps)
                _evac(on_act, dup0[0:64, sl], ps[64:128, :])
                _evac(on_act, dup0[64:128, sl], ps[0:64, :])
                ps = psproj.tile([128, 512], F32, name="psk2", tag="psp")
                for k in range(5):
                    nc.tensor.matmul(
                        ps[:64, :],
                        wk_sb[k][:, 128:192],
                        xt_sb[k][:, sl],
                        start=(k == 0), stop=(k == 4),
                    )
                _evac(on_act, kt1[:, sl], ps[0:64, :])
                _evac(on_act, dup1[64:128, sl], ps[0:64, :])

            def q_pass(c, qb2, on_act=False):
                m0, mp = MCHUNKS[c]
                qg = qs_list[2 * qb2]
                ps = psproj.tile([128, 512], F32, name="psq", tag="psp")
                for k in range(5):
                    nc.tensor.matmul(
                        ps[:mp, :],
                        wq_sb[k][:, m0:m0 + mp],
                        xt_sb[k][:, qg:qg + 512],
                        start=(k == 0), stop=(k == 4),
                    )
                _evac(on_act, qt_sb[c][:, qb2 * 512:(qb2 + 1) * 512], ps[:mp, :])

            def v_pass(sblk, on_act=False):
                psf = psproj.tile([128, 512], F32, name="psv", tag="psp")
                ps = psf[:, 0:NKV * HD]
                for k in range(5):
                    nc.tensor.matmul(
                        ps,
                        xt_sb[k][:, sblk * 128:(sblk + 1) * 128],
                        wv_sb[k],
                        start=(k == 0), stop=(k == 4),
                    )
                _evac(
                    on_act,
                    vall[:, sblk, :, 0:HD],
                    ps.rearrange("p (h d) -> p h d", h=NKV),
                )

            # phase 0: only what superblocks 0/1 need; rest becomes tasks
            # interleaved into the attention loop to fill PE idle time.
            # Q-proj chunk c is emitted just before superblock 0's pair c.
            # phase-0 passes in xt arrival order
            for nb in xt_order:
                if nb < p0_knb:
                    k_pass(nb, on_act=True)
                    for sblk in range(nb * 4, min(nb * 4 + 4, p0_vsb)):
                        v_pass(sblk, on_act=True)
            k_done = [nb < p0_knb for nb in range(nnb)]
            v_done = [sblk < p0_vsb for sblk in range(nkb)]
            # remaining passes ordered by the first kb block that needs them
            tasks = sorted(
                [("k", nb, nb * 4) for nb in range(p0_knb, nnb)]
                + [("v", sblk, sblk) for sblk in range(p0_vsb, nkb)],
                key=lambda t: (t[2], t[0]),
            )

            def emit_task(kind, arg):
                if kind == "k":
                    if not k_done[arg]:
                        k_done[arg] = True
                        k_pass(arg)
                else:
                    if not v_done[arg]:
                        v_done[arg] = True
                        v_pass(arg)

            def need_kb(kb):
                # forced deadline: K/V data for key block kb must exist before
                # the scores/attn@V matmuls that read it are emitted
                emit_task("k", kb // 4)
                emit_task("v", kb)
                while tasks and tasks[0][2] <= kb:
                    kind, arg, _ = tasks.pop(0)
                    emit_task(kind, arg)

            # KT slice for kv head at a given array-half (base 0 or 64)
            def kt_src(kv, base):
                if base == 0:
                    return [kt0[0:64, :], dup0[0:64, :], kt1[0:64, :]][kv]
                return [dup0[64:128, :], kt0[64:128, :], dup1[64:128, :]][kv]

            # attnT chunks [128, 1024] (+ ones row on chunk 4 for bo folding)
            at_sb = []
            for c in range(4):
                t = attnp.tile([128, SQ], BF16, name=f"at{c}", tag=f"at{c}")
                at_sb.append(t)
            t = attnp.tile([65, SQ], BF16, name="at4", tag="at4")
            nc.vector.memset(t[64:65, :], 1.0)
            at_sb.append(t)

            # ---- attention + per-superblock output projection ----
            # The attn@V matmuls for group g are emitted after the scores of
            # group g+1 (software pipelining): the PE queue is strict FIFO,
            # so emitting attn@V right after its own scores would head-block
            # the queue while the exp runs.
            def emit_attnv(st):
                # one accumulation chain per pair tile: PSUM groups are
                # bank-granular (start clears has_written for the whole bank),
                # so the two heads cannot run separate start/stop chains in
                # the shared [65, 512] bank.
                sb, pair, g, ot, wt, nkbs = st
                np_ = len(pair)
                ngrp = nkbs // G
                for i, h in enumerate(pair):
                    for j in range(G):
                        kb = g * G + j
                        nc.tensor.matmul(
                            ot[:, i * SB:(i + 1) * SB],
                            vall[:, kb, kv_of(h), :],
                            wt[:, (i * G + j) * SB:(i * G + j + 1) * SB],
                            start=(g == 0 and i == 0 and j == 0),
                            stop=(g == ngrp - 1 and i == np_ - 1 and j == G - 1),
                        )

            def emit_norm(st):
                sb, pair, g, ot, wt, nkbs = st
                np_ = len(pair)
                w = np_ * SB
                rec = smallp.tile([1, 512], F32, name="rec", tag="rec")
                nc.vector.reciprocal(rec[:, 0:w], ot[64:65, 0:w])
                bc = smallp.tile([64, 512], F32, name="bc", tag="bc")
                if no_bcast:
                    nc.vector.memset(bc, 1.0)
                else:
                    nc.gpsimd.partition_broadcast(bc[:, 0:w], rec[:, 0:w])
                for i, h in enumerate(pair):
                    r0 = (h % 2) * 64
                    nc.vector.tensor_mul(
                        at_sb[h // 2][r0:r0 + 64, sb * SB:(sb + 1) * SB],
                        ot[0:64, i * SB:(i + 1) * SB],
                        bc[:, i * SB:(i + 1) * SB],
                    )

            def flush(st):
                if st is None:
                    return
                emit_attnv(st)
                if st[2] == st[5] // G - 1:  # last group of its pair
                    emit_norm(st)

            pending = None
            for si, sb in enumerate(sb_order):
                qs = qs_list[sb]
                nkbs = qs // 128 + 2
                ngrp = nkbs // G
                for pi, pair in enumerate(PAIRS):
                    np_ = len(pair)
                    if si == 0:
                        q_pass(pi, 0, on_act=True)
                        q_pass(pi, 1, on_act=True)
                    ot = psot.tile([65, 512], F32, name="ot", tag="ot")
                    qt_c = qt_sb[pair[0] // 2]
                    for g in range(ngrp):
                        need_kb(g * G + G - 1)
                        ps = pssc.tile([128, 2 * G * SB], F32, name="pssc",
                                       tag="pssc")
                        wt = wtp.tile([128, 2 * G * SB], BF16, name="wt",
                                      tag="wt")
                        for i, h in enumerate(pair):
                            base = (h % 2) * 64
                            lhs = kt_src(kv_of(h), base)
                            rhs = qt_c[base:base + 64, sb * SB:(sb + 1) * SB]
                            for j in range(G):
                                kb = g * G + j
                                nc.tensor.matmul(
                                    ps[:, (i * G + j) * SB:(i * G + j + 1) * SB],
                                    lhs[:, kb * 128:(kb + 1) * 128],
                                    rhs,
                                    start=True, stop=True,
                                    tile_position=(base, 0),
                                )
                        if g == 0 and si > 0 and tasks:
                            kind, arg, _ = tasks.pop(0)
                            emit_task(kind, arg)
                        flush(pending)
                        w = np_ * G * SB
                        # exp(scores/8), psum -> sbuf bf16
                        nc.scalar.activation(
                            wt[:, 0:w], ps[:, 0:w],
                            func=mybir.ActivationFunctionType.Exp,
                            scale=0.125,
                        )
                        # causal mask on the last group's diagonal blocks
                        if g == ngrp - 1 and not no_afsel:
                            nc.vector.tensor_mul(
                                wt[:, 0:w], wt[:, 0:w], maskp[:, 0:w]
                            )
                        pending = (sb, pair, g, ot, wt, nkbs)
                flush(pending)
                pending = None
                # ---- output projection for this superblock's q rows ----
                for qb in range(2 * sb, 2 * sb + 2):
                    ott = outp.tile([128, DM], BF16, name="outt", tag="outt")
                    for half in range(2):
                        pso = psot.tile([128, 288], F32, name="pso", tag="ot")
                        for c in range(5):
                            nc.tensor.matmul(
                                pso,
                                at_sb[c][:, qb * 128:(qb + 1) * 128],
                                wo_sb[c][:, half * 288:(half + 1) * 288],
                                start=(c == 0), stop=(c == 4),
                            )
                        # late superblocks evacuate on ACT (exp stream has
                        # dried up there; DVE is the tail bottleneck)
                        _evac(si >= 2, ott[:, half * 288:(half + 1) * 288], pso)
                    nc.sync.dma_start(out_d[qb * 128:(qb + 1) * 128, :], ott)

    nc.compile()
    return nc


_cache = {}


def _programs():
    if "A" not in _cache:
        _cache["A"] = build_program(TYPE_A["qs"], TYPE_A["skv"])
        _cache["B"] = build_program(TYPE_B["qs"], TYPE_B["skv"])
    return _cache["A"], _cache["B"]


def _make_runner(nc, devices):
    """Cached shard_map runner for `nc` pinned to an explicit device subset.

    Mirrors concourse.bass2jax.run_bass_via_pjrt's multi-core branch, but
    with a persistent jit and a caller-chosen device list so two programs
    can run concurrently on disjoint NeuronCore subsets.
    """
    import jax
    from jax.experimental.shard_map import shard_map
    from jax.sharding import Mesh, PartitionSpec
    from concourse import bass2jax, mybir as _mb

    bass2jax.install_neuronx_cc_hook()
    n_cores = len(devices)

    part_name = nc.partition_id_tensor.name if nc.partition_id_tensor else None
    in_names, out_names, out_avals = [], [], []
    for alloc in nc.m.functions[0].allocations:
        if not isinstance(alloc, mybir.MemoryLocationSet):
            continue
        name = alloc.memorylocations[0].name
        if alloc.kind == "ExternalInput":
            if name != part_name:
                in_names.append(name)
        elif alloc.kind == "ExternalOutput":
            out_names.append(name)
            out_avals.append(
                jax.core.ShapedArray(tuple(alloc.tensor_shape), _mb.dt.np(alloc.dtype))
            )
    n_params = len(in_names)
    n_outs = len(out_avals)
    all_names = in_names + out_names + ([part_name] if part_name else [])
    donate = tuple(range(n_params, n_params + n_outs))

    def _body(*args):
        args = list(args)
        if part_name:
            args.append(bass2jax.partition_id_tensor())
        outs = bass2jax._bass_exec_p.bind(
            *args,
            out_avals=tuple(out_avals),
            in_names=tuple(all_names),
            out_names=tuple(out_names),
            lowering_input_output_aliases=(),
            sim_require_finite=True,
            sim_require_nnan=True,
            nc=nc,
        )
        return tuple(outs)

    mesh = Mesh(np.asarray(devices), ("core",))
    in_specs = (PartitionSpec("core"),) * (n_params + n_outs)
    out_specs = (PartitionSpec("core"),) * n_outs
    sharded = jax.jit(
        shard_map(_body, mesh=mesh, in_specs=in_specs, out_specs=out_specs,
                  check_rep=False),
        donate_argnums=donate, keep_unused=True,
    )

    def run(in_maps, block=True):
        concat_in = [
            np.concatenate([np.asarray(m[name]) for m in in_maps], axis=0)
            for name in in_names
        ]
        zeros = [
            np.zeros((n_cores * a.shape[0], *a.shape[1:]), a.dtype) for a in out_avals
        ]
        out_arrs = sharded(*concat_in, *zeros)
        def collect():
            return [
                {name: np.asarray(out_arrs[i]).reshape(n_cores, *out_avals[i].shape)[c]
                 for i, name in enumerate(out_names)}
                for c in range(n_cores)
            ]
        return collect() if block else collect

    return run


def _runners():
    if "runA" not in _cache:
        import jax
        devs = jax.devices()
        ncA, ncB = _programs()
        _cache["runA"] = _make_runner(ncA, devs[0:4])
        _cache["runB"] = _make_runner(ncB, devs[4:8])
    return _cache["runA"], _cache["runB"]


def _host_inputs(inputs, Wq, bq, Wk, bk, Wv, bv, Wo, bo):
    bf = ml_dtypes.bfloat16
    x = np.asarray(inputs, dtype=np.float32)
    wq = np.vstack([np.asarray(Wq, np.float32), np.asarray(bq, np.float32)[None]]).astype(bf)
    wk = np.vstack([np.asarray(Wk, np.float32), np.asarray(bk, np.float32)[None]]).astype(bf)
    wv = np.vstack([np.asarray(Wv, np.float32), np.asarray(bv, np.float32)[None]]).astype(bf)
    wo = np.vstack([np.asarray(Wo, np.float32), np.asarray(bo, np.float32)[None]]).astype(bf)
    xts = []
    for b in range(B):
        xt = np.empty((DME, S), bf)
        xt[:DM] = x[b].T.astype(bf)
        xt[DM] = 1.0
        xts.append(np.ascontiguousarray(xt))
    return xts, wq, wk, wv, wo


def kernel(inputs, Wq, bq, Wk, bk, Wv, bv, Wo, bo):
    xts, wq, wk, wv, wo = _host_inputs(inputs, Wq, bq, Wk, bk, Wv, bv, Wo, bo)
    ones = np.ones((128, 9), np.float32)
    maps_a = [dict(xt=xts[b], wq=wq, wk=wk, wv=wv, wo=wo, ones=ones)
              for b in range(B)]
    maps_b = [dict(xt=np.ascontiguousarray(xts[b][:, :TYPE_B["skv"]]), wq=wq,
                   wk=wk, wv=wv, wo=wo, ones=ones) for b in range(B)]
    try:
        run_a, run_b = _runners()
        col_a = run_a(maps_a, block=False)
        col_b = run_b(maps_b, block=False)
        res_a, res_b = col_a(), col_b()
    except Exception:
        res_a, res_b = _kernel_fallback(maps_a, maps_b)

    out = np.empty((B, S, DM), np.float32)
    for b in range(B):
        oa = np.asarray(res_a[b]["out"], np.float32)
        ob = np.asarray(res_b[b]["out"], np.float32)
        out[b, 0:512] = oa[0:512]
        out[b, 1536:2048] = oa[512:1024]
        out[b, 512:1536] = ob
    return out


def _kernel_fallback(maps_a, maps_b):
    ncA, ncB = _programs()
    res_a = run_bass_kernel_spmd(ncA, maps_a, core_ids=[0, 1, 2, 3]).results
    res_b = run_bass_kernel_spmd(ncB, maps_b, core_ids=[0, 1, 2, 3]).results
    return res_a, res_b
